# revision 19
# baseline (speedup 1.0000x reference)
"""AutoCorrelation layer kernel for 8 Trainium2 NeuronCores.

Math note: the reference's rfft/irfft pair over the zero-padded head dim
computes a circular cross-correlation; its mean over all lags collapses
analytically to (sum_d q_proj) * (sum_d k_proj) per head.  So
corr_mean[b,l] = (1/(H*L)) * sum_h (q[b,l] @ WqS)_h * (k[b,l] @ WkS)_h
with WqS = Wq.reshape(D,H,DK).sum(-1).  Everything downstream (top-6,
softmax, gather, output projection) follows the reference directly.
(bq/bk are zero in this problem; host asserts that and the device skips
them.  bv and bp are applied: bv on device, bp on the host.)

Distribution: all 8 cores redundantly run the cheap preprocessing with
f32 q/k (the top-6/7 corr gap is only 1.2e-5 in batch 1, so sub-f32
q/k precision can flip the selected set), and each core computes its
own column shard of the (256, 262144) output projection from a
host-pre-cast bf16 copy of Wp (halves the dominant HBM stream vs f32).
v is loaded bf16 (only enters the weighted value average).
"""
import sys

sys.path.insert(0, "/opt/trn_rl_repo")

import numpy as np
import concourse.bass as bass
import concourse.mybir as mybir
import concourse.tile as tile
from concourse import bacc
from concourse.bass_utils import run_bass_kernel_spmd
from concourse.masks import make_identity

F32 = mybir.dt.float32
F32R = mybir.dt.float32r
BF16 = mybir.dt.bfloat16

N_CORES = 8
B, L, D, H, DK = 8, 1024, 256, 8, 32
K_TOP = 6
NSH = (L * D) // N_CORES          # 32768 output cols per core
CHUNK = 2048                      # big-matmul tile (cols)
N_CHUNKS = NSH // CHUNK           # 16
SCALE = 1.0 / (H * L)

WP_BUFS = 8
TRACE = False          # test harness sets this for profiled runs
LAST_RESULT = None     # stashed BassKernelResults from the last kernel() call

_CACHE = {}


def _build_nc():
    nc = bacc.Bacc("TRN2", target_bir_lowering=False, debug=False, num_devices=N_CORES)

    qt_d = nc.dram_tensor("qt", [128, 2 * B * L], F32R, kind="ExternalInput").ap()
    kt_d = nc.dram_tensor("kt", [128, 2 * B * L], F32R, kind="ExternalInput").ap()
    v_d = nc.dram_tensor("v", [128, B * 8 * D], BF16, kind="ExternalInput").ap()
    wqs_d = nc.dram_tensor("wqs", [128, 2 * H], F32R, kind="ExternalInput").ap()
    wks_d = nc.dram_tensor("wks", [128, 2 * H], F32R, kind="ExternalInput").ap()
    wv_d = nc.dram_tensor("wv", [128, 2 * D], F32, kind="ExternalInput").ap()
    bvt_d = nc.dram_tensor("bvt", [128, 2], F32, kind="ExternalInput").ap()
    blk3_d = nc.dram_tensor("blk3", [8, 8 * 8], F32R, kind="ExternalInput").ap()
    bdm_d = nc.dram_tensor("bdm", [8, B * 64], F32, kind="ExternalInput").ap()
    wp_d = nc.dram_tensor("wp", [128, N_CHUNKS * 2 * CHUNK], BF16, kind="ExternalInput").ap()
    out_d = nc.dram_tensor("out", [B, NSH], BF16, kind="ExternalOutput").ap()

    qt_v = qt_d.rearrange("p (c b l) -> p c b l", c=2, b=B)
    kt_v = kt_d.rearrange("p (c b l) -> p c b l", c=2, b=B)
    v_v = v_d.rearrange("p (b t e) -> p b t e", b=B, t=8)
    wp_v = wp_d.rearrange("p (n c m) -> p n c m", n=N_CHUNKS, c=2)

    with tile.TileContext(nc) as tc:
        with (
            tc.tile_pool(name="cst", bufs=1) as cst,
            tc.tile_pool(name="qk", bufs=1) as qk,
            tc.tile_pool(name="work", bufs=2) as work,
            tc.tile_pool(name="wpp", bufs=WP_BUFS) as wpp,
            tc.tile_pool(name="outp", bufs=3) as outp,
            tc.tile_pool(name="px", bufs=2, space="PSUM") as px,
            tc.tile_pool(name="ps_r", bufs=1, space="PSUM") as ps_rp,
            tc.tile_pool(name="ps_big", bufs=2, space="PSUM") as ps_big,
        ):
            # ---- constants (scalar/ACT HWDGE ring, parallel with bulk loads) ----
            ident8 = cst.tile([8, 8], F32)
            make_identity(nc, ident8[:, :])
            wqs_sb = cst.tile([128, 2, H], F32R)
            nc.scalar.dma_start(wqs_sb[:, :, :], wqs_d.rearrange("p (c h) -> p c h", c=2))
            wks_sb = cst.tile([128, 2, H], F32R)
            nc.scalar.dma_start(wks_sb[:, :, :], wks_d.rearrange("p (c h) -> p c h", c=2))
            wv_sb = cst.tile([128, 2, D], F32)
            nc.scalar.dma_start(wv_sb[:, :, :], wv_d.rearrange("p (c d) -> p c d", c=2))
            bvt_sb = cst.tile([128, 2], F32)
            nc.scalar.dma_start(bvt_sb[:, :], bvt_d)
            # blk3[h, b, m] = SCALE * (m == b): per-batch selector for the
            # corr reduction over heads (host-provided structural constant)
            blk3 = cst.tile([8, 8, 8], F32R)
            nc.scalar.dma_start(blk3[:, :, :], blk3_d.rearrange("h (b m) -> h b m", b=8))
            # bdm[m, (b, e)] = (b == m): block-diagonal extraction mask
            bdm = cst.tile([8, B * 64], F32)
            nc.scalar.dma_start(bdm[:, :], bdm_d)

            # ---- bulk inputs (sync HWDGE ring, in priority order) ----
            v_sb = qk.tile([128, B, 8, D], BF16)
            nc.scalar.dma_start(v_sb[:, :, :, :], v_v)

            # ---- per-batch q/k head-sum projections -> corr rows ----
            ps_r = ps_rp.tile([8, L], F32, tag="r")
            for b in range(B):
                qt_b = work.tile([128, 2, L], F32R, tag="qtb", bufs=3)
                nc.sync.dma_start(qt_b[:, :, :], qt_v[:, :, b, :])
                kt_b = work.tile([128, 2, L], F32R, tag="ktb", bufs=3)
                nc.sync.dma_start(kt_b[:, :, :], kt_v[:, :, b, :])
                prods = []
                for half in range(2):
                    sl = slice(512 * half, 512 * (half + 1))
                    ps_qs = px.tile([8, 512], F32, tag="px")
                    nc.tensor.matmul(ps_qs[:, :], wqs_sb[:, 0, :],
                                     qt_b[:, 0, sl], start=True, stop=False)
                    nc.tensor.matmul(ps_qs[:, :], wqs_sb[:, 1, :],
                                     qt_b[:, 1, sl], start=False, stop=True)
                    ps_ks = px.tile([8, 512], F32, tag="px")
                    nc.tensor.matmul(ps_ks[:, :], wks_sb[:, 0, :],
                                     kt_b[:, 0, sl], start=True, stop=False)
                    nc.tensor.matmul(ps_ks[:, :], wks_sb[:, 1, :],
                                     kt_b[:, 1, sl], start=False, stop=True)
                    qs_sb = work.tile([8, 512], F32, tag="qs", bufs=3)
                    nc.scalar.copy(qs_sb[:, :], ps_qs[:, :])
                    prod = work.tile([8, 512], F32R, tag="prod", bufs=3)
                    nc.vector.tensor_mul(prod[:, :], ps_ks[:, :], qs_sb[:, :])
                    prods.append((sl, prod))
                for sl, prod in prods:
                    nc.tensor.matmul(ps_r[:, sl], blk3[:, b, :],
                                     prod[:, :],
                                     start=(b == 0), stop=(b == B - 1))

            # ---- top-6 via top-8, masked softmax over all l ----
            r_sb = cst.tile([8, L], F32)
            nc.vector.tensor_copy(r_sb[:, :], ps_r[:, :])
            topv = cst.tile([8, 8], F32)
            nc.vector.max(topv[:, :], r_sb[:, :])
            negm = cst.tile([8, 1], F32)
            nc.vector.tensor_scalar_mul(negm[:, :], topv[:, 0:1], -1.0)
            e_sb = cst.tile([8, L], F32)
            nc.scalar.activation(e_sb[:, :], r_sb[:, :],
                                 mybir.ActivationFunctionType.Exp,
                                 bias=negm[:, 0:1], scale=1.0)
            mask = cst.tile([8, L], F32)
            nc.vector.tensor_scalar(
                out=mask[:, :], in0=r_sb[:, :],
                scalar1=topv[:, K_TOP - 1:K_TOP], scalar2=None,
                op0=mybir.AluOpType.is_ge)
            ew = cst.tile([8, L], F32)
            nc.vector.tensor_mul(ew[:, :], e_sb[:, :], mask[:, :])
            z_sb = cst.tile([8, 1], F32)
            nc.vector.reduce_sum(out=z_sb[:, :], in_=ew[:, :], axis=mybir.AxisListType.X)
            zinv = cst.tile([8, 1], F32)
            nc.vector.reciprocal(zinv[:, :], z_sb[:, :])
            selw = cst.tile([8, L], F32)
            nc.vector.tensor_scalar_mul(selw[:, :], ew[:, :], zinv[:, 0:1])

            # ---- selT[p, t, b] = selw[b, 128t + p] (bf16) ----
            selT = cst.tile([128, 8, 8], BF16)
            for t in range(8):
                tp8 = px.tile([128, 8], F32, tag="px")
                nc.tensor.transpose(tp8[:, :], selw[0:8, 128 * t:128 * (t + 1)], ident8[:, :])
                nc.vector.tensor_copy(selT[:, t, :], tp8[:, :])

            # ---- vbar[b, e] = sum_l selw[b, l] v[b, l, e] (diag of PE result) ----
            vbar = cst.tile([8, D], F32)
            for e4 in range(4):
                ps_vb = ps_big.tile([8, B * 64], F32, tag="big")
                for t in range(8):
                    nc.tensor.matmul(ps_vb[:, :], selT[:, t, :],
                                     v_sb[:, :, t, 64 * e4:64 * (e4 + 1)],
                                     start=(t == 0), stop=(t == 7))
                vmask = work.tile([8, B * 64], F32, tag="vmask", bufs=2)
                nc.vector.tensor_mul(vmask[:, :], ps_vb[:, :], bdm[:, :])
                nc.vector.reduce_sum(
                    out=vbar[:, 64 * e4:64 * (e4 + 1)],
                    in_=vmask[:, :].rearrange("p (b2 e) -> p e b2", b2=B),
                    axis=mybir.AxisListType.X)

            # ---- aggT[d', b] = sum_e Wv[e, d'] vbar[b, e] + bv[d'] (bf16) ----
            vbarT = cst.tile([128, 2, 8], F32)
            for c in range(2):
                tpv = px.tile([128, 8], F32, tag="px")
                nc.tensor.transpose(tpv[:, :], vbar[0:8, 128 * c:128 * (c + 1)], ident8[:, :])
                nc.vector.tensor_copy(vbarT[:, c, :], tpv[:, :])
            aggt_bf = cst.tile([128, 2, 8], BF16)
            for m in range(2):
                ps_a = px.tile([128, 8], F32, tag="px")
                nc.tensor.matmul(ps_a[:, :], wv_sb[:, 0, 128 * m:128 * (m + 1)],
                                 vbarT[:, 0, :], start=True, stop=False)
                nc.tensor.matmul(ps_a[:, :], wv_sb[:, 1, 128 * m:128 * (m + 1)],
                                 vbarT[:, 1, :], start=False, stop=True)
                nc.vector.tensor_scalar(
                    out=aggt_bf[:, m, :], in0=ps_a[:, :],
                    scalar1=bvt_sb[:, m:m + 1], scalar2=None,
                    op0=mybir.AluOpType.add)

            # ---- big output projection (column shard), bf16 stream ----
            for n in range(N_CHUNKS):
                wp_sb = wpp.tile([128, 2, CHUNK], BF16, tag="wp")
                nc.sync.dma_start(wp_sb[:, :, :], wp_v[:, n, :, :])
                o_sb = outp.tile([8, CHUNK], BF16)
                for j in range(2):
                    ps = ps_big.tile([8, 1024], F32, tag="big")
                    for s in range(2):
                        sl = slice(1024 * j + 512 * s, 1024 * j + 512 * (s + 1))
                        pl = slice(512 * s, 512 * (s + 1))
                        nc.tensor.matmul(ps[:, pl], aggt_bf[:, 0, :], wp_sb[:, 0, sl],
                                         start=True, stop=False)
                        nc.tensor.matmul(ps[:, pl], aggt_bf[:, 1, :], wp_sb[:, 1, sl],
                                         start=False, stop=True)
                    osl = slice(1024 * j, 1024 * (j + 1))
                    if (2 * n + j) % 2 == 0:
                        nc.scalar.copy(o_sb[:, osl], ps[:, :])
                    else:
                        nc.vector.tensor_copy(o_sb[:, osl], ps[:, :])
                nc.scalar.dma_start(out_d[:, CHUNK * n:CHUNK * (n + 1)], o_sb[:, :])

    nc.finalize()
    return nc


def _get_nc():
    if "nc" not in _CACHE:
        _CACHE["nc"] = _build_nc()
    return _CACHE["nc"]


def _pack_inputs(queries, keys, values, Wq, Wk, Wv, bv, Wp):
    """Host-side layout prep shared by all cores (wp/bp shards differ)."""
    bf = np.dtype("bfloat16") if hasattr(np, "bfloat16") else None
    import ml_dtypes
    bf16 = ml_dtypes.bfloat16

    def to_bf(x):
        return np.ascontiguousarray(x.astype(bf16))

    # q/k transposed per batch: [128, 2, B, L] with d = c*128 + p
    def pack_qk(x):
        # x: [B, L, D] -> [p, c, b, l]
        t = x.reshape(B, L, 2, 128).transpose(3, 2, 0, 1)
        return np.ascontiguousarray(t).reshape(128, 2 * B * L).astype(np.float32)

    qt = pack_qk(queries)
    kt = pack_qk(keys)
    # v: [B, L, D] -> [p, b, t, e] with l = t*128 + p
    vt = values.reshape(B, 8, 128, D).transpose(2, 0, 1, 3)
    vp = to_bf(np.ascontiguousarray(vt).reshape(128, B * 8 * D))
    # head-sum projection weights [D, H] -> [p, c, h]
    wqs = Wq.reshape(D, H, DK).sum(-1)
    wks = Wk.reshape(D, H, DK).sum(-1)

    def pack_ws(w):
        t = w.reshape(2, 128, H).transpose(1, 0, 2)
        return np.ascontiguousarray(t).reshape(128, 2 * H).astype(np.float32)

    wv_p = np.ascontiguousarray(
        Wv.reshape(2, 128, D).transpose(1, 0, 2).reshape(128, 2 * D).astype(np.float32))
    bvt = np.ascontiguousarray(bv.reshape(2, 128).T.astype(np.float32))
    blk3 = np.zeros((8, 8, 8), np.float32)
    for b in range(B):
        blk3[:, b, b] = SCALE
    return {
        "qt": qt, "kt": kt, "v": vp,
        "wqs": pack_ws(wqs), "wks": pack_ws(wks),
        "wv": wv_p, "bvt": bvt,
        "blk3": np.ascontiguousarray(blk3.reshape(8, 64)),
        "bdm": np.ascontiguousarray(
            np.repeat(np.eye(8, dtype=np.float32), 64, axis=1)),
    }


def kernel(queries, keys, values, Wq, bq, Wk, bk, Wv, bv, Wp, bp):
    import ml_dtypes
    bf16 = ml_dtypes.bfloat16

    queries = np.asarray(queries, np.float32)
    keys = np.asarray(keys, np.float32)
    values = np.asarray(values, np.float32)
    Wq = np.asarray(Wq, np.float32)
    Wk = np.asarray(Wk, np.float32)
    Wv = np.asarray(Wv, np.float32)
    Wp = np.asarray(Wp, np.float32)
    bq = np.asarray(bq, np.float32)
    bk = np.asarray(bk, np.float32)
    bv = np.asarray(bv, np.float32)
    bp = np.asarray(bp, np.float32)
    assert not bq.any() and not bk.any(), "kernel assumes zero q/k biases"

    common = _pack_inputs(queries, keys, values, Wq, Wk, Wv, bv, Wp)

    nc = _get_nc()
    in_maps = []
    for i in range(N_CORES):
        cols = slice(NSH * i, NSH * (i + 1))
        # wp shard packed [p, n, c, m]: wp[c*128+p, n*CHUNK+m]
        shard = Wp[:, cols].reshape(2, 128, N_CHUNKS, CHUNK).transpose(1, 2, 0, 3)
        m = dict(common)
        m["wp"] = np.ascontiguousarray(shard.astype(bf16)).reshape(128, N_CHUNKS * 2 * CHUNK)
        in_maps.append(m)

    res = run_bass_kernel_spmd(nc, in_maps, core_ids=list(range(N_CORES)), trace=TRACE)
    global LAST_RESULT
    LAST_RESULT = res
    outs = [res.results[i]["out"].astype(np.float32) for i in range(N_CORES)]
    out = np.concatenate(outs, axis=1) + bp[None, :]
    return out.reshape(B, L, D)
